# revision 1
# baseline (speedup 1.0000x reference)
"""ContextQueryAttention Trainium2 Bass kernel.

Full-input contract: kernel(context[64,1024,128], query[64,128,128],
W[384,1], query_mask[64,128]) -> out[64,1024,512] (f32).

Sharding: data-parallel over batch B across 8 NeuronCores (8 batches/core).

Per-core design (fp32r matmuls, 256-wide streams):
  - context[b] loaded as [p, t, d] with c = 8p + t (contiguous 4KB/partition)
  - S_tile[c, 0:128] = s_term, col 128 = c_term, via one fp32r matmul with
    rhs = [qT*w_s | w_c | pad-to-256]; q_term+mask row added on DVE via a
    PE-broadcast tile
  - softmax over q: DVE rowmax(negate) + ACT Exp; row sum fused into the c2q
    matmul as an extra ones column
  - q2c: global-over-C softmax via transpose-max trick + partition-sum
    matmul; q2c computed in row form (lhsT = eM column, rhs = ctx)
  - output: cols 0:128 stored straight from the ctx tile; cols 128:512
    assembled in a staging tile
"""

import sys

import numpy as np

try:
    import concourse.bass as bass  # noqa: F401
except ImportError:  # grading dir may lack the site config
    sys.path.insert(0, "/opt/trn_rl_repo")

import concourse.bass as bass
import concourse.mybir as mybir
import concourse.tile as tile
from concourse import bacc
from concourse.bass_utils import run_bass_kernel_spmd
from concourse.masks import make_identity

F32 = mybir.dt.float32
F32R = mybir.dt.float32r
P = 128          # partitions
D = 128          # feature dim
Q = 128          # query len
C = 1024         # context len
CT = C // P      # context tiles per batch
N_CORES = 8
B_FULL = 64
B_SHARD = B_FULL // N_CORES  # 8 batches per core
W_PAD = 256      # fp32r fast path needs moving free dim >= 256


def build_program(n_batches: int = B_SHARD) -> bass.Bass:
    # Bacc (not raw Bass): its compile() runs move_matmul_waits_to_ldweights,
    # required because walrus allows only one sync-wait per PE instruction.
    nc = bacc.Bacc(None, target_bir_lowering=False)

    ctx_d = nc.declare_dram_parameter("context", [n_batches, C, D], F32, isOutput=False)
    qry_d = nc.declare_dram_parameter("query", [n_batches, Q, D], F32, isOutput=False)
    w_d = nc.declare_dram_parameter("W", [3 * D, 1], F32, isOutput=False)
    msk_d = nc.declare_dram_parameter("query_mask", [n_batches, Q], F32, isOutput=False)
    out_d = nc.declare_dram_parameter("out", [n_batches, C, 4 * D], F32, isOutput=True)

    with tile.TileContext(nc) as tc:
        with (
            tc.tile_pool(name="singles", bufs=1) as singles,
            tc.tile_pool(name="ctxp", bufs=2) as ctxp,
            tc.tile_pool(name="stp", bufs=2) as stp,
            tc.tile_pool(name="bp", bufs=2) as bp,
            tc.tile_pool(name="tp", bufs=3) as tp,
            tc.tile_pool(name="sp", bufs=3) as sp,
            tc.tile_pool(name="ps_tp", bufs=3, space="PSUM") as ps_tp,
            tc.tile_pool(name="ps_w", bufs=3, space="PSUM") as ps_w,
            tc.tile_pool(name="ps_q2c", bufs=1, space="PSUM") as ps_q2c,
            tc.tile_pool(name="ps_sm", bufs=1, space="PSUM") as ps_sm,
        ):
            # ---- one-time constants ----
            identity_f = singles.tile([P, P], F32)
            make_identity(nc, identity_f)
            identity = singles.tile([P, P], F32R)
            nc.vector.tensor_copy(out=identity, in_=identity_f)
            # memset can't write f32r tiles; build f32 scratch and round-copy
            onesP_f = singles.tile([P, P], F32)
            nc.vector.memset(onesP_f, 1.0)
            onesP = singles.tile([P, P], F32R)
            nc.vector.tensor_copy(out=onesP, in_=onesP_f)
            zeroP_f = singles.tile([P, W_PAD - Q - 1], F32)
            nc.vector.memset(zeroP_f, 0.0)
            zeroP = singles.tile([P, W_PAD - Q - 1], F32R)
            nc.vector.tensor_copy(out=zeroP, in_=zeroP_f)

            # W [384,1] -> wvec [128,3] (cols: w_c, w_q, w_s)
            w3 = singles.tile([3, P], F32)
            nc.sync.dma_start(out=w3, in_=w_d.rearrange("(g d) o -> g (d o)", g=3))
            wv_ps = ps_sm.tile([P, 512], F32, tag="small")
            nc.tensor.transpose(wv_ps[:, 0:3], w3, identity_f[:3, :3])
            wvec = singles.tile([P, 3], F32R)
            nc.scalar.copy(wvec, wv_ps[:, 0:3])

            # full query_mask as a single row [1, n_batches*Q]
            msk_row = singles.tile([1, n_batches * Q], F32)
            nc.sync.dma_start(out=msk_row, in_=msk_d.rearrange("b q -> (b q)")[None, :])

            for b in range(n_batches):
                # ---- loads ----
                ctx_sb = ctxp.tile([P, CT, D], F32, tag="ctx")
                nc.sync.dma_start(
                    out=ctx_sb, in_=ctx_d[b].rearrange("(p t) d -> p t d", t=CT)
                )
                qry_sb = bp.tile([Q, D], F32, tag="qry")
                nc.sync.dma_start(out=qry_sb, in_=qry_d[b])

                # out cols 0:128 = context, straight from the load tile
                nc.sync.dma_start(
                    out=out_d[b].rearrange("(p t) d -> p t d", t=CT)[:, :, 0:D],
                    in_=ctx_sb,
                )

                # ---- per-batch prep ----
                # rounded copies for fp32r matmuls
                ctx_r = ctxp.tile([P, CT, D], F32R, tag="ctxr")
                nc.vector.tensor_copy(out=ctx_r, in_=ctx_sb)
                # rhs for c2q: [query | ones | pad]; col 128 of the product
                # gives the softmax denominator for free
                rhs_cq = bp.tile([Q, W_PAD], F32R, tag="rhscq")
                nc.vector.tensor_copy(out=rhs_cq[:, 0:D], in_=qry_sb)
                nc.vector.tensor_copy(out=rhs_cq[:, D:], in_=onesP[:, : W_PAD - D])

                qT_ps = ps_tp.tile([P, P], F32R, tag="tp")
                nc.tensor.transpose(qT_ps, rhs_cq[:, 0:D], identity)  # [d, q]
                qT_sb = bp.tile([P, Q], F32R, tag="qT")
                nc.scalar.copy(qT_sb, qT_ps)

                # rhs for S: [qT * w_s | w_c | pad]
                rhs_s = bp.tile([P, W_PAD], F32R, tag="rhss")
                nc.vector.tensor_scalar_mul(
                    rhs_s[:, 0:Q], qT_sb, wvec[:, 2:3].bitcast(F32)
                )
                nc.gpsimd.tensor_copy(out=rhs_s[:, Q + 1 :], in_=zeroP)
                nc.gpsimd.tensor_copy(out=rhs_s[:, Q : Q + 1], in_=wvec[:, 0:1])

                # q_term[q] = sum_d qT[d,q] * w_q[d]  -> [1, Q] (psum)
                small_ps = ps_sm.tile([P, 512], F32, tag="small")
                nc.tensor.matmul(small_ps[0:1, 0:Q], lhsT=wvec[:, 1:2], rhs=qT_sb)

                # qrow = q_term + (1-mask)*NEG_INF
                mb_sb = bp.tile([1, Q], F32, tag="mb")
                nc.vector.tensor_scalar(
                    mb_sb,
                    msk_row[:, b * Q : (b + 1) * Q],
                    1e9,
                    -1e9,
                    op0=mybir.AluOpType.mult,
                    op1=mybir.AluOpType.add,
                )
                qrow_sb = bp.tile([1, Q], F32, tag="qrow")
                nc.vector.tensor_add(qrow_sb, small_ps[0:1, 0:Q], mb_sb)
                # broadcast qrow to all partitions: ones[1,P].T @ qrow[1,Q]
                qbc_ps = ps_tp.tile([P, Q], F32, tag="tp")
                nc.tensor.matmul(qbc_ps, lhsT=onesP_f[0:1, :], rhs=qrow_sb)
                qbc = bp.tile([P, Q], F32, tag="qbc")
                nc.scalar.copy(qbc, qbc_ps)

                Mcols = bp.tile([P, CT], F32, tag="Mcols")
                stage = stp.tile([P, CT, 3 * D], F32, tag="stage")

                for i in range(CT):
                    ctx_i = ctx_sb[:, i, :]
                    # ctxT = transpose(ctx_r_i) : [d, c]
                    ctxT_ps = ps_tp.tile([P, P], F32R, tag="tp")
                    nc.tensor.transpose(ctxT_ps, ctx_r[:, i, :], identity)
                    ctxT_sb = tp.tile([P, P], F32R, tag="ctxT")
                    if i % 2 == 0:
                        nc.vector.tensor_copy(out=ctxT_sb, in_=ctxT_ps)
                    else:
                        nc.scalar.copy(ctxT_sb, ctxT_ps)

                    # wide psum holds S in [:, 0:256] and c2q in [:, 256:512]
                    wide_ps = ps_w.tile([P, 512], F32, tag="wide")
                    # S: cols 0:128 s_term, col 128 c_term, cols 129:256 junk
                    nc.tensor.matmul(wide_ps[:, 0:W_PAD], lhsT=ctxT_sb, rhs=rhs_s)

                    # Spq = S + qrow (broadcast); mn = -rowmax(Spq)
                    Spq_sb = tp.tile([P, Q], F32, tag="Spq")
                    mn = sp.tile([P, 1], F32, tag="mn")
                    nc.vector.tensor_add(Spq_sb, wide_ps[:, 0:Q], qbc)
                    nc.vector.reduce_max(
                        mn, Spq_sb, axis=mybir.AxisListType.X, negate=True
                    )
                    # M[c] = c_term[c] + rowmax = c_term - mn
                    nc.vector.tensor_sub(
                        Mcols[:, i : i + 1], wide_ps[:, Q : Q + 1], mn
                    )

                    # e = exp(Spq - rowmax)
                    e_sb = tp.tile([P, Q], F32R, tag="e")
                    nc.scalar.activation(
                        e_sb,
                        Spq_sb,
                        mybir.ActivationFunctionType.Exp,
                        bias=mn,
                        scale=1.0,
                    )

                    # c2q_unnorm = (e.T).T @ [query | ones]; col 128 = sumexp
                    eT_ps = ps_tp.tile([P, P], F32R, tag="tp")
                    nc.tensor.transpose(eT_ps, e_sb, identity)
                    eT_sb = tp.tile([P, P], F32R, tag="eT")
                    if i % 2 == 0:
                        nc.scalar.copy(eT_sb, eT_ps)
                    else:
                        nc.vector.tensor_copy(out=eT_sb, in_=eT_ps)
                    nc.tensor.matmul(
                        wide_ps[:, 256 : 256 + W_PAD], lhsT=eT_sb, rhs=rhs_cq
                    )

                    r_col = sp.tile([P, 1], F32, tag="r")
                    nc.vector.reciprocal(r_col, wide_ps[:, 256 + D : 256 + D + 1])
                    # stage: [c2q | ctx*c2q | ctx*q2c]
                    nc.scalar.mul(stage[:, i, 0:D], wide_ps[:, 256 : 256 + D], r_col)
                    if i % 2 == 0:
                        nc.vector.tensor_mul(
                            stage[:, i, D : 2 * D], ctx_i, stage[:, i, 0:D]
                        )
                    else:
                        nc.gpsimd.tensor_mul(
                            stage[:, i, D : 2 * D], ctx_i, stage[:, i, 0:D]
                        )

                # ---- q2c: softmax over all C of M, then weighted sum of ctx ----
                rmax_col = sp.tile([P, 1], F32, tag="rmax")
                nc.vector.reduce_max(rmax_col, Mcols, axis=mybir.AxisListType.X)
                nc.tensor.transpose(small_ps[0:1, 128:256], rmax_col, identity_f)
                neg_g = sp.tile([1, 1], F32, tag="negg")
                nc.vector.reduce_max(
                    neg_g, small_ps[0:1, 128:256], axis=mybir.AxisListType.X, negate=True
                )
                neg_gc_ps = ps_tp.tile([P, 1], F32, tag="tp")
                nc.tensor.matmul(neg_gc_ps, lhsT=onesP_f[0:1, :], rhs=neg_g)
                neg_g_col = sp.tile([P, 1], F32, tag="neggc")
                nc.vector.tensor_copy(out=neg_g_col, in_=neg_gc_ps)

                eM = bp.tile([P, CT], F32R, tag="eM")
                rowsum = sp.tile([P, 1], F32, tag="rowsum")
                nc.scalar.activation(
                    eM,
                    Mcols,
                    mybir.ActivationFunctionType.Exp,
                    bias=neg_g_col,
                    accum_out=rowsum,
                )
                # T = sum over partitions of rowsum
                nc.tensor.matmul(
                    small_ps[0:1, 384:385], lhsT=rowsum, rhs=onesP_f[:, 0:1]
                )
                rT = sp.tile([1, 1], F32, tag="rT")
                nc.vector.reciprocal(rT, small_ps[0:1, 384:385])

                # q2c row: accumulate lhsT=eM[:,i] (1-col weights), rhs=ctx_r
                q2c_ps = ps_q2c.tile([1, D], F32, tag="q2c")
                for i in range(CT):
                    nc.tensor.matmul(
                        q2c_ps,
                        lhsT=eM[:, i : i + 1],
                        rhs=ctx_r[:, i, :],
                        start=(i == 0),
                        stop=(i == CT - 1),
                    )
                q2c_row = bp.tile([1, D], F32, tag="q2crow")
                nc.scalar.mul(q2c_row, q2c_ps, rT)
                q2cbc_ps = ps_tp.tile([P, D], F32, tag="tp")
                nc.tensor.matmul(q2cbc_ps, lhsT=onesP_f[0:1, :], rhs=q2c_row)
                q2c_bc = bp.tile([P, D], F32, tag="q2cbc")
                nc.scalar.copy(q2c_bc, q2cbc_ps)

                for i in range(CT):
                    nc.gpsimd.tensor_mul(
                        stage[:, i, 2 * D : 3 * D], ctx_sb[:, i, :], q2c_bc
                    )

                # ---- store cols 128:512 ----
                nc.sync.dma_start(
                    out=out_d[b].rearrange("(p t) d -> p t d", t=CT)[:, :, D:],
                    in_=stage,
                )

    nc.compile()
    return nc


_CACHED = {}


def _get_program(n_batches: int = B_SHARD) -> bass.Bass:
    if n_batches not in _CACHED:
        _CACHED[n_batches] = build_program(n_batches)
    return _CACHED[n_batches]


def kernel(context, query, W, query_mask, **run_kwargs):
    context = np.ascontiguousarray(np.asarray(context, dtype=np.float32))
    query = np.ascontiguousarray(np.asarray(query, dtype=np.float32))
    W = np.ascontiguousarray(np.asarray(W, dtype=np.float32))
    query_mask = np.ascontiguousarray(np.asarray(query_mask, dtype=np.float32))

    nc = _get_program(B_SHARD)
    in_maps = []
    for c in range(N_CORES):
        s = slice(c * B_SHARD, (c + 1) * B_SHARD)
        in_maps.append(
            {
                "context": np.ascontiguousarray(context[s]),
                "query": np.ascontiguousarray(query[s]),
                "W": W,
                "query_mask": np.ascontiguousarray(query_mask[s]),
            }
        )
    res = run_bass_kernel_spmd(nc, in_maps, core_ids=list(range(N_CORES)), **run_kwargs)
    out = np.concatenate([r["out"] for r in res.results], axis=0)
    if run_kwargs:
        kernel.last_result = res
    return out



# revision 17
# speedup vs baseline: 1.0044x; 1.0044x over previous
"""ContextQueryAttention Trainium2 Bass kernel (v2).

Full-input contract: kernel(context[64,1024,128], query[64,128,128],
W[384,1], query_mask[64,128]) -> out[64,1024,512] (f32).

Sharding: data-parallel over batch B across 8 NeuronCores (8 batches/core).

v2 design notes (per batch, c = p*8 + t layout):
  - c_term is fused into the S matmul rhs: rhs_s = qT*w_s + w_c, so the
    row max of (S + qrow) directly gives M[c] for the q2c path.
  - ctxT transposes write into the junk half of the S psum tile (the
    256-wide fp32r fast path wastes cols 128:256 anyway).
  - softmax is 4 batch-wide ops: add(+qbc), rowmax(neg), add(-M), one
    1024-wide exp on ACT; Z by one DVE reduce_sum over the bf16 e.
  - e/eT/c2q run in bf16 (1 cycle/row at any width on PE).
  - f32 tiles are bitcast to f32r for matmul inputs (no CAST copies).
  - 2-stage software pipeline: stage1(b+1) PE work overlaps stage2(b).
  - output staged as full [128, 8, 512] rows -> one 16KB/partition
    contiguous store, split in two dma_starts.
"""

import sys

import numpy as np

try:
    import concourse.bass as bass  # noqa: F401
except ImportError:  # grading dir may lack the site config
    sys.path.insert(0, "/opt/trn_rl_repo")

import concourse.bass as bass
import concourse.mybir as mybir
import concourse.tile as tile
from concourse import bacc
from concourse.bass_utils import run_bass_kernel_spmd
from concourse.masks import make_identity

F32 = mybir.dt.float32
F32R = mybir.dt.float32r
BF16 = mybir.dt.bfloat16
P = 128          # partitions
D = 128          # feature dim
Q = 128          # query len
C = 1024         # context len
CT = C // P      # context tiles per batch (8)
N_CORES = 8
B_FULL = 64
B_SHARD = B_FULL // N_CORES  # 8 batches per core
W_PAD = 256      # fp32r fast path needs moving/out free dim >= 256


def build_program(n_batches: int = B_SHARD) -> bass.Bass:
    nc = bacc.Bacc(None, target_bir_lowering=False)

    ctx_d = nc.declare_dram_parameter("context", [n_batches, C, D], F32, isOutput=False)
    qry_d = nc.declare_dram_parameter("query", [n_batches, Q, D], F32, isOutput=False)
    w_d = nc.declare_dram_parameter("W", [3 * D, 1], F32, isOutput=False)
    msk_d = nc.declare_dram_parameter("query_mask", [n_batches, Q], F32, isOutput=False)
    out_d = nc.declare_dram_parameter("out", [n_batches, C, 4 * D], F32, isOutput=True)

    with tile.TileContext(nc) as tc:
        with (
            tc.tile_pool(name="singles", bufs=1) as singles,
            tc.tile_pool(name="ctxp", bufs=3) as ctxp,
            tc.tile_pool(name="qryp", bufs=3) as qryp,
            tc.tile_pool(name="bp", bufs=2) as bp,
            tc.tile_pool(name="sp", bufs=2) as sp,
            tc.tile_pool(name="spqp", bufs=2) as spqp,
            tc.tile_pool(name="ep", bufs=2) as ep,
            tc.tile_pool(name="tp", bufs=2) as tp,
            tc.tile_pool(name="stp", bufs=2) as stp,
            tc.tile_pool(name="ps_s", bufs=1, space="PSUM") as ps_s,
            tc.tile_pool(name="ps_misc", bufs=1, space="PSUM") as ps_misc,
            tc.tile_pool(name="ps_et", bufs=1, space="PSUM") as ps_et,
            tc.tile_pool(name="ps_c2q", bufs=1, space="PSUM") as ps_c2q,
        ):
            # ---- one-time constants ----
            identity_f = singles.tile([P, P], F32)
            make_identity(nc, identity_f)
            identity_bf = singles.tile([P, P], BF16)
            nc.vector.tensor_copy(out=identity_bf, in_=identity_f)
            ones_col = singles.tile([P, 1], F32)
            nc.vector.memset(ones_col, 1.0)
            ones_r = singles.tile([P, 1], F32R)
            nc.vector.tensor_copy(out=ones_r, in_=ones_col)
            # eM buffer padded to 2 cols/tile (zeros) for fp32r even-width
            # ISA rules; zero cols written once here, col 0 per batch by ACT
            eM2 = singles.tile([P, CT, 2], F32R)
            zscr = singles.tile([P, CT, 2], F32)
            nc.vector.memset(zscr, 0.0)
            nc.vector.tensor_copy(out=eM2, in_=zscr)

            # W [384,1] -> wvec_f [128,3] (cols: w_c, w_q, w_s)
            w3 = singles.tile([3, P], F32)
            nc.sync.dma_start(out=w3, in_=w_d.rearrange("(g d) o -> g (d o)", g=3))
            wv_ps = ps_misc.tile([P, 512], F32, tag="misc")
            nc.tensor.transpose(wv_ps[:, 0:3], w3, identity_f[:3, :3])
            wvec_f = singles.tile([P, 3], F32)
            nc.scalar.copy(wvec_f, wv_ps[:, 0:3])
            wvec_r = singles.tile([P, 3], F32R)
            nc.vector.tensor_copy(out=wvec_r, in_=wv_ps[:, 0:3])

            # maskterm row: (1-mask)*NEG_INF for all batches
            msk_row = singles.tile([1, n_batches * Q], F32)
            nc.sync.dma_start(out=msk_row, in_=msk_d.rearrange("b q -> (b q)")[None, :])
            maskterm = singles.tile([1, n_batches * Q], F32)
            nc.vector.tensor_scalar(
                maskterm, msk_row, 1e9, -1e9,
                op0=mybir.AluOpType.mult, op1=mybir.AluOpType.add,
            )

            state = {}

            def stage1(b):
                st = {}
                # ---- loads ----
                ctx_sb = ctxp.tile([P, CT, D], F32, tag="ctx")
                nc.sync.dma_start(
                    out=ctx_sb, in_=ctx_d[b].rearrange("(p t) d -> p t d", t=CT)
                )
                qry_sb = qryp.tile([Q, D], F32, tag="qry")
                nc.sync.dma_start(out=qry_sb, in_=qry_d[b])
                qry_bf = qryp.tile([Q, D], BF16, tag="qrybf")
                nc.vector.tensor_copy(out=qry_bf, in_=qry_sb)

                misc_ps = ps_misc.tile([P, 512], F32, tag="misc")

                # qT = transpose(query): [d, q]
                nc.tensor.transpose(misc_ps[:, 0:128], qry_sb, identity_f)
                qT_sb = bp.tile([P, W_PAD], F32R, tag="qT")
                nc.scalar.copy(qT_sb[:, 0:128], misc_ps[:, 0:128])

                # rhs_s = qT*w_s + w_c  (c_term folded in); right half junk
                rhs_s = bp.tile([P, W_PAD], F32R, tag="rhss")
                nc.vector.tensor_scalar(
                    rhs_s[:, 0:128], qT_sb[:, 0:128],
                    wvec_f[:, 2:3], wvec_f[:, 0:1],
                    op0=mybir.AluOpType.mult, op1=mybir.AluOpType.add,
                )

                # q_term[q] = sum_d qT[d,q]*w_q[d] -> [1, 256] (cols 128: junk)
                nc.tensor.matmul(
                    misc_ps[0:1, 256:512],
                    lhsT=wvec_r[:, 1:2],
                    rhs=qT_sb,
                )
                qrow_sb = bp.tile([1, Q], F32, tag="qrow")
                nc.vector.tensor_add(
                    qrow_sb, misc_ps[0:1, 256:384],
                    maskterm[:, b * Q:(b + 1) * Q],
                )
                qbc = bp.tile([P, Q], F32, tag="qbc")
                nc.gpsimd.partition_broadcast(qbc, qrow_sb)

                # ctxT transposes into the junk half of the S psum tile
                s_ps = ps_s.tile([P, CT, W_PAD], F32, tag="s")
                for t in range(CT):
                    nc.tensor.transpose(
                        s_ps[:, t, 128:256], ctx_sb[:, t, :], identity_f
                    )
                ctxT_sb = tp.tile([P, CT, D], F32R, tag="ctxT")
                nc.scalar.copy(ctxT_sb, s_ps[:, :, 128:256])

                # S matmuls (overwrite full 256-wide rows incl. junk half)
                for t in range(CT):
                    nc.tensor.matmul(
                        s_ps[:, t, :],
                        lhsT=ctxT_sb[:, t, :],
                        rhs=rhs_s,
                    )

                # softmax prep: spq = S + qrow; negM = -rowmax; spq2 = spq-M
                spq = spqp.tile([P, CT, Q], F32, tag="spq")
                nc.vector.tensor_add(
                    spq, s_ps[:, :, 0:128],
                    qbc[:, :].unsqueeze(1).broadcast_to((P, CT, Q)),
                )
                negM = sp.tile([P, CT], F32, tag="negM")
                nc.vector.reduce_max(
                    negM, spq, axis=mybir.AxisListType.X, negate=True
                )
                spq2 = spqp.tile([P, CT, Q], F32, tag="spq2")
                nc.vector.tensor_add(
                    spq2, spq,
                    negM[:, :].unsqueeze(2).broadcast_to((P, CT, Q)),
                )
                e_all = ep.tile([P, CT, Q], BF16, tag="e")
                nc.scalar.activation(
                    e_all, spq2, mybir.ActivationFunctionType.Exp
                )
                Z = sp.tile([P, CT], F32, tag="Z")
                nc.vector.tensor_reduce(
                    Z, e_all, axis=mybir.AxisListType.X, op=mybir.AluOpType.add
                )
                r_all = sp.tile([P, CT], F32, tag="r")
                nc.vector.reciprocal(r_all, Z)

                st.update(
                    ctx_sb=ctx_sb, qry_bf=qry_bf, misc_ps=misc_ps,
                    e_all=e_all, negM=negM, r_all=r_all,
                )
                return st

            def stage2(b, st):
                ctx_sb = st["ctx_sb"]
                e_all = st["e_all"]
                negM = st["negM"]
                r_all = st["r_all"]
                misc_ps = st["misc_ps"]
                qry_bf = st["qry_bf"]

                stage = stp.tile([P, CT, 4 * D], F32, tag="stage")
                nc.gpsimd.tensor_copy(out=stage[:, :, 0:D], in_=ctx_sb)
                # rounded ctx copy for the fp32r q2c accumulation
                ctx_r = tp.tile([P, CT, D], F32R, tag="ctxr")
                nc.gpsimd.tensor_copy(out=ctx_r, in_=ctx_sb)

                # eT transposes (bf16) + copy to sbuf
                et_ps = ps_et.tile([P, CT, Q], BF16, tag="et")
                for t in range(CT):
                    nc.tensor.transpose(
                        et_ps[:, t, :], e_all[:, t, :], identity_bf
                    )
                eT_all = tp.tile([P, CT, Q], BF16, tag="eT")
                nc.vector.tensor_copy(out=eT_all, in_=et_ps)

                # c2q matmuls (bf16): out[c, d] per tile
                c2q_ps = ps_c2q.tile([P, CT, D], F32, tag="c2q")
                for t in range(CT):
                    nc.tensor.matmul(
                        c2q_ps[:, t, :], lhsT=eT_all[:, t, :], rhs=qry_bf
                    )

                # stage cols 128:256 = c2q_unnorm * r  (6 ACT / 2 DVE)
                for t in range(CT):
                    if t % 4 == 3:
                        nc.vector.tensor_scalar_mul(
                            stage[:, t, D:2 * D], c2q_ps[:, t, :],
                            r_all[:, t:t + 1],
                        )
                    else:
                        nc.scalar.mul(
                            stage[:, t, D:2 * D], c2q_ps[:, t, :],
                            r_all[:, t:t + 1],
                        )
                # stage cols 256:384 = ctx * c2q
                nc.gpsimd.tensor_mul(
                    stage[:, :, 2 * D:3 * D], ctx_sb, stage[:, :, D:2 * D]
                )

                # ---- q2c ----
                nm_col = sp.tile([P, 1], F32, tag="nm")
                nc.vector.tensor_reduce(
                    nm_col, negM, axis=mybir.AxisListType.X, op=mybir.AluOpType.min
                )
                nc.tensor.transpose(misc_ps[0:1, 128:256], nm_col, identity_f)
                ng = sp.tile([1, 1], F32, tag="ng")
                nc.vector.tensor_reduce(
                    ng, misc_ps[0:1, 128:256], axis=mybir.AxisListType.X,
                    op=mybir.AluOpType.min,
                )
                ng_col = sp.tile([P, 1], F32, tag="ngc")
                nc.gpsimd.partition_broadcast(ng_col, ng)
                # eM = exp(M - g) = exp(-negM + ng) into col 0 of eM2
                nc.scalar.activation(
                    eM2[:, :, 0], negM, mybir.ActivationFunctionType.Exp,
                    bias=ng_col, scale=-1.0,
                )
                # T = total sum of eM: partition-sum -> [1, 2*CT], then reduce
                nc.tensor.matmul(
                    misc_ps[0:1, 496:512], lhsT=ones_r, rhs=eM2[:, :, :],
                )
                Tsum = sp.tile([1, 1], F32, tag="Tsum")
                nc.vector.tensor_reduce(
                    Tsum, misc_ps[0:1, 496:512], axis=mybir.AxisListType.X,
                    op=mybir.AluOpType.add,
                )
                rT = sp.tile([1, 1], F32, tag="rT")
                nc.vector.reciprocal(rT, Tsum)
                # q2cT[d] = sum_t sum_c ctx[c,t,d]*eM[c,t] (psum accumulate)
                for t in range(CT):
                    nc.tensor.matmul(
                        misc_ps[:, 252:254],
                        lhsT=ctx_r[:, t, :],
                        rhs=eM2[:, t, :],
                        start=(t == 0), stop=(t == CT - 1),
                    )
                q2c_col = sp.tile([P, 1], F32, tag="q2ccol")
                nc.vector.tensor_copy(out=q2c_col, in_=misc_ps[:, 252:253])
                nc.tensor.transpose(
                    misc_ps[0:1, 256:384], q2c_col, identity_f
                )
                q2c_row = bp.tile([1, D], F32, tag="q2crow")
                nc.vector.tensor_scalar_mul(q2c_row, misc_ps[0:1, 256:384], rT)
                q2c_bc = bp.tile([P, D], F32, tag="q2cbc")
                nc.gpsimd.partition_broadcast(q2c_bc, q2c_row)
                # stage cols 384:512 = ctx * q2c
                nc.vector.tensor_mul(
                    stage[:, :, 3 * D:4 * D], ctx_sb,
                    q2c_bc[:, :].unsqueeze(1).broadcast_to((P, CT, D)),
                )

                # ---- store (two 8KB/partition contiguous chunks) ----
                out_ap = out_d[b].rearrange("(p t) d -> p t d", t=CT)
                nc.sync.dma_start(out=out_ap[:, 0:CT // 2, :], in_=stage[:, 0:CT // 2, :])
                nc.sync.dma_start(out=out_ap[:, CT // 2:, :], in_=stage[:, CT // 2:, :])

            for b in range(n_batches + 1):
                if b < n_batches:
                    state[b] = stage1(b)
                if b > 0:
                    stage2(b - 1, state.pop(b - 1))

    nc.compile()
    return nc


_CACHED = {}


def _get_program(n_batches: int = B_SHARD) -> bass.Bass:
    if n_batches not in _CACHED:
        _CACHED[n_batches] = build_program(n_batches)
    return _CACHED[n_batches]


def kernel(context, query, W, query_mask, **run_kwargs):
    context = np.ascontiguousarray(np.asarray(context, dtype=np.float32))
    query = np.ascontiguousarray(np.asarray(query, dtype=np.float32))
    W = np.ascontiguousarray(np.asarray(W, dtype=np.float32))
    query_mask = np.ascontiguousarray(np.asarray(query_mask, dtype=np.float32))

    nc = _get_program(B_SHARD)
    in_maps = []
    for c in range(N_CORES):
        s = slice(c * B_SHARD, (c + 1) * B_SHARD)
        in_maps.append(
            {
                "context": np.ascontiguousarray(context[s]),
                "query": np.ascontiguousarray(query[s]),
                "W": W,
                "query_mask": np.ascontiguousarray(query_mask[s]),
            }
        )
    res = run_bass_kernel_spmd(nc, in_maps, core_ids=list(range(N_CORES)), **run_kwargs)
    out = np.concatenate([r["out"] for r in res.results], axis=0)
    if run_kwargs:
        kernel.last_result = res
    return out
